# revision 9
# baseline (speedup 1.0000x reference)
"""NeighborAttention (GNN message passing) Trainium2 kernel.

Contract: kernel(**inputs) takes FULL unsharded inputs (as produced by
reference.setup_inputs()) and returns the FULL [10000, 256] output.

Strategy (8 NeuronCores, SPMD, no collectives):
  center_id is sorted, so partitioning nodes also partitions edges into
  contiguous ranges. Core k owns nodes [1280k, 1280k+1280) and exactly the
  edges whose center falls in that range. All segment ops (softmax over
  edges sharing a center, weighted scatter-sum) are then core-local.

  Within a core: 10 groups of 128 nodes. Each group's edges are packed on
  the host into T tiles of 512 edges (T = max over all groups, data
  dependent; zero-padded). Per 512-edge tile, feature-major MLP chains run
  on the tensor engine in float32r (full-rate fp32 with 11-bit mantissa).
  The segment softmax is restructured as a single pass:
      out[n] = (sum_e p_e * v_e) / (sum_e p_e),   p_e = exp(logit_e)
  where both sums accumulate in PSUM across the group's tiles via matmuls
  with a one-hot assignment matrix A[e, n] = (center(e) == n) generated
  on-device (iota + is_equal).  exp is computed via tanh so that every
  activation (Relu / Gelu / Tanh / Copy) lives in one ACT table set.
"""

import os
import sys
import types

for _p in ("/opt/trn_rl_repo", "/root/.axon_site"):
    if _p not in sys.path and os.path.isdir(_p):
        sys.path.insert(0, _p)

import numpy as np

import concourse.bass as bass
import concourse.tile as tile
from concourse import bacc, mybir
from concourse import bass_utils
from concourse.bass import ts

F32 = mybir.dt.float32
F32R = mybir.dt.float32r
AF = mybir.ActivationFunctionType
ALU = mybir.AluOpType

NCORES = 8
N_NODES = 10000
N_EDGES = 300000
H = 256            # hidden
NIN = 512          # edge feature dim
HEADS = 4
D = 64             # head dim
NODE_BLOCK = 128   # nodes per group (one PSUM partition block)
GROUPS = 10        # groups per core
NODES_PER_CORE = NODE_BLOCK * GROUPS   # 1280 (8*1280 = 10240 >= 10000)
TILE_E = 512       # edges per tile
SUBT = TILE_E // 128  # 4 subtiles of 128 edges


def _round_f32r(a: np.ndarray) -> np.ndarray:
    """Round fp32 to the fp32r grid (11-bit mantissa, round half up) so the
    on-device fp32r matmuls see exactly these values."""
    b = np.ascontiguousarray(a, dtype=np.float32).view(np.uint32)
    b = ((b + 0x800) & np.uint32(0xFFFFF000)).astype(np.uint32)
    return b.view(np.float32).reshape(a.shape)


def _build_program(T: int, G: int) -> "bacc.Bacc":
    EG = T * TILE_E            # padded edges per window
    EPACK = G * EG             # padded edges per core
    NPACK = G * NODE_BLOCK     # windowed node columns (with duplication)

    nc = bacc.Bacc("TRN2", target_bir_lowering=False, debug=False,
                   enable_asserts=True, num_devices=NCORES)

    dram = lambda n, s, dt: nc.dram_tensor(n, s, dt, kind="ExternalInput").ap()
    hET = dram("hET", [NIN, EPACK], F32R)          # packed h_E, transposed
    crel = dram("crel", [EPACK], F32R)             # center - group base; pad -1
    hVT = dram("hVT", [H, NPACK], F32R)   # h_V.T windowed per group
    b1we = dram("b1we", [NIN, H], F32R)            # b1_w rows 256:768 (edge part)
    b1wv = dram("b1wv", [H, H], F32R)              # b1_w rows 0:256 (node part)
    b2w = dram("b2w", [H, H], F32R)
    b3ws = dram("b3ws", [H, HEADS], F32R)          # b3_w / sqrt(d)
    wv1 = dram("wv1", [NIN, H], F32R)
    wv2 = dram("wv2", [H, H], F32R)
    wv3 = dram("wv3", [H, H], F32R)
    wv3bc = dram("wv3bc", [128, 2], F32)
    wow = dram("wow", [H, H], F32R)
    b1b = dram("b1b", [128, 2], F32)
    b2b = dram("b2b", [128, 2], F32)
    wv1b = dram("wv1b", [128, 2], F32)
    wv2b = dram("wv2b", [128, 2], F32)
    iota_chan = dram("iota_chan", [128, TILE_E], F32)  # [p, j] = p
    iota_row = dram("iota_row", [128, 128], F32)       # [p, j] = j
    ident = dram("ident", [128, 128], F32)

    out_d = nc.dram_tensor("out", [NPACK, H], F32,
                           kind="ExternalOutput").ap()

    with tile.TileContext(nc) as tc:
        with tc.tile_pool(name="wc", bufs=1) as wc, \
             tc.tile_pool(name="he", bufs=4) as hep, \
             tc.tile_pool(name="act", bufs=3) as actp, \
             tc.tile_pool(name="sm", bufs=4) as smp, \
             tc.tile_pool(name="grp", bufs=2) as grpp, \
             tc.tile_pool(name="psmlp", bufs=3, space="PSUM") as psmlp, \
             tc.tile_pool(name="psmisc", bufs=1, space="PSUM") as psmisc, \
             tc.tile_pool(name="psve", bufs=2, space="PSUM") as psve, \
             tc.tile_pool(name="psacc", bufs=2, space="PSUM") as psacc:

            # ---- load weights / constants (once) ----
            b1we_sb = wc.tile([128, 4, H], F32R)
            nc.sync.dma_start(b1we_sb[:], b1we.rearrange("(c p) m -> p c m", p=128))
            b1wv_sb = wc.tile([128, 2, H], F32R)
            nc.sync.dma_start(b1wv_sb[:], b1wv.rearrange("(c p) m -> p c m", p=128))
            b2w_sb = wc.tile([128, 2, H], F32R)
            nc.sync.dma_start(b2w_sb[:], b2w.rearrange("(c p) m -> p c m", p=128))
            b3ws_sb = wc.tile([128, 2, HEADS], F32R)
            nc.sync.dma_start(b3ws_sb[:], b3ws.rearrange("(c p) m -> p c m", p=128))
            wv1_sb = wc.tile([128, 4, H], F32R)
            nc.sync.dma_start(wv1_sb[:], wv1.rearrange("(c p) m -> p c m", p=128))
            wv2_sb = wc.tile([128, 2, H], F32R)
            nc.sync.dma_start(wv2_sb[:], wv2.rearrange("(c p) m -> p c m", p=128))
            wv3_sb = wc.tile([128, 2, H], F32R)
            nc.sync.dma_start(wv3_sb[:], wv3.rearrange("(c p) m -> p c m", p=128))
            wv3bc_sb = wc.tile([128, 2], F32)
            nc.sync.dma_start(wv3bc_sb[:], wv3bc[:])
            wow_sb = wc.tile([128, 2, H], F32R)
            nc.sync.dma_start(wow_sb[:], wow.rearrange("(c p) m -> p c m", p=128))
            hVT_sb = wc.tile([128, 2, NPACK], F32R)
            nc.sync.dma_start(hVT_sb[:], hVT.rearrange("(c p) n -> p c n", p=128))
            b1b_sb = wc.tile([128, 2], F32)
            nc.sync.dma_start(b1b_sb[:], b1b[:])
            b2b_sb = wc.tile([128, 2], F32)
            nc.sync.dma_start(b2b_sb[:], b2b[:])
            wv1b_sb = wc.tile([128, 2], F32)
            nc.sync.dma_start(wv1b_sb[:], wv1b[:])
            wv2b_sb = wc.tile([128, 2], F32)
            nc.sync.dma_start(wv2b_sb[:], wv2b[:])
            ichan_sb = wc.tile([128, TILE_E], F32)
            nc.sync.dma_start(ichan_sb[:], iota_chan[:])
            irow_sb = wc.tile([128, 128], F32)
            nc.sync.dma_start(irow_sb[:], iota_row[:])
            ident_sb = wc.tile([128, 128], F32)
            nc.sync.dma_start(ident_sb[:], ident[:])

            # ---- u = h_V @ b1_w[:256]  (node-major [1280, 256]) ----
            u_sb = wc.tile([128, G, H], F32R)
            for nch in range(G):
                up = psmisc.tile([128, H], F32, tag="misc")
                for c in range(2):
                    nc.tensor.matmul(up[:], hVT_sb[:, c, ts(nch, 128)],
                                     b1wv_sb[:, c, :],
                                     start=(c == 0), stop=(c == 1))
                nc.scalar.copy(u_sb[:, nch, :], up[:])

            # ---- main loop over groups / tiles ----
            for g in range(G):
                acc = psacc.tile([128, 512], F32)  # [:,0:256] sum p*v, [:,256:260] sum p

                # per-subtile center columns: transpose [T*4, 128] -> [128, T*4]
                ctile = grpp.tile([128, 128], F32)
                nc.sync.dma_start(
                    ctile[:4 * T, :],
                    crel[g * EG:(g + 1) * EG].bitcast(F32).rearrange(
                        "(s e) -> s e", e=128))
                ctp = psmisc.tile([128, 128], F32, tag="misc")
                nc.tensor.transpose(ctp[:, :4 * T], ctile[:4 * T, :],
                                    ident_sb[:4 * T, :4 * T])
                ccols = grpp.tile([128, 4 * T], F32)
                nc.vector.tensor_copy(ccols[:], ctp[:, :4 * T])

                for t in range(T):
                    e0 = g * EG + t * TILE_E
                    first = (t == 0)
                    last = (t == T - 1)

                    het = hep.tile([128, 4, TILE_E], F32R)
                    nc.sync.dma_start(
                        het[:], hET[:, e0:e0 + TILE_E].rearrange(
                            "(c p) e -> p c e", p=128))

                    # broadcast center over partitions: cb[p, e] = c[e]
                    cb = smp.tile([128, TILE_E], F32, tag="cb")
                    nc.sync.dma_start(
                        cb[:], crel.bitcast(F32)[e0:e0 + TILE_E][None, :]
                        .to_broadcast([128, TILE_E]))
                    an = smp.tile([128, TILE_E], F32R)  # A_n[n, e] = (n == c[e])
                    nc.vector.tensor_tensor(an[:], ichan_sb[:], cb[:],
                                            op=ALU.is_equal)

                    # bias-MLP layer 1: x1 = relu(b1we.T @ hE + u[center] + b1b)
                    x1sb = actp.tile([128, 2, TILE_E], F32R, tag="x1")
                    for m in range(2):
                        x1p = psmlp.tile([128, TILE_E], F32, tag="mlp")
                        for c in range(4):
                            nc.tensor.matmul(x1p[:], b1we_sb[:, c, ts(m, 128)],
                                             het[:, c, :],
                                             start=(c == 0), stop=False)
                        nc.tensor.matmul(x1p[:], u_sb[:, g, ts(m, 128)], an[:],
                                         start=False, stop=True)
                        nc.scalar.activation(x1sb[:, m, :], x1p[:], AF.Relu,
                                             bias=b1b_sb[:, m:m + 1], scale=1.0)

                    # W_V MLP
                    v1sb = actp.tile([128, 2, TILE_E], F32R, tag="v1")
                    for m in range(2):
                        v1p = psmlp.tile([128, TILE_E], F32, tag="mlp")
                        for c in range(4):
                            nc.tensor.matmul(v1p[:], wv1_sb[:, c, ts(m, 128)],
                                             het[:, c, :],
                                             start=(c == 0), stop=(c == 3))
                        nc.scalar.activation(v1sb[:, m, :], v1p[:], AF.Gelu,
                                             bias=wv1b_sb[:, m:m + 1], scale=1.0)

                    # bias-MLP layer 2
                    x2sb = actp.tile([128, 2, TILE_E], F32R, tag="x2")
                    for m in range(2):
                        x2p = psmlp.tile([128, TILE_E], F32, tag="mlp")
                        for c in range(2):
                            nc.tensor.matmul(x2p[:], b2w_sb[:, c, ts(m, 128)],
                                             x1sb[:, c, :],
                                             start=(c == 0), stop=(c == 1))
                        nc.scalar.activation(x2sb[:, m, :], x2p[:], AF.Relu,
                                             bias=b2b_sb[:, m:m + 1], scale=1.0)

                    # logits (edge-major): w_e[e, h] per subtile; lhsT = x2 chunk
                    v2sb = actp.tile([128, 2, TILE_E], F32R, tag="v2")
                    for m in range(2):
                        v2p = psmlp.tile([128, TILE_E], F32, tag="mlp")
                        for c in range(2):
                            nc.tensor.matmul(v2p[:], wv2_sb[:, c, ts(m, 128)],
                                             v2_in_c := v1sb[:, c, :],
                                             start=(c == 0), stop=(c == 1))
                        nc.scalar.activation(v2sb[:, m, :], v2p[:], AF.Gelu,
                                             bias=wv2b_sb[:, m:m + 1], scale=1.0)

                    # single start/stop per PSUM bank: start marks the whole
                    # 2KB zero-region pending-zero, so only the bank's first
                    # matmul may carry start=True.
                    wep = psve.tile([128, H], F32, tag="ve")
                    for s in range(SUBT):
                        for c in range(2):
                            nc.tensor.matmul(wep[:, 4 * s:4 * s + 4],
                                             x2sb[:, c, ts(s, 128)],
                                             b3ws_sb[:, c, :],
                                             start=(s == 0 and c == 0),
                                             stop=(s == SUBT - 1 and c == 1))
                    # p = exp(l) = (1+tanh(l/2)) / (1-tanh(l/2))
                    th = smp.tile([128, 16], F32, tag="th")
                    nc.scalar.activation(th[:], wep[:, :16], AF.Tanh, scale=0.5)
                    pnum = smp.tile([128, 16], F32, tag="pnum")
                    nc.vector.tensor_scalar(pnum[:], th[:], 1.0, None, ALU.add)
                    pden = smp.tile([128, 16], F32, tag="pden")
                    nc.vector.tensor_scalar(pden[:], th[:], -1.0, 1.0,
                                            ALU.mult, ALU.add)
                    prec = smp.tile([128, 16], F32, tag="prec")
                    nc.vector.reciprocal(prec[:], pden[:])
                    psb = smp.tile([128, 16], F32R, tag="psb")
                    nc.vector.tensor_tensor(psb[:], pnum[:], prec[:], op=ALU.mult)


                    # per 128-edge subtile: v_e (edge-major), weight by p, scatter
                    for s in range(SUBT):
                        vep = psve.tile([128, H], F32, tag="ve")
                        for c in range(2):
                            nc.tensor.matmul(vep[:], v2sb[:, c, ts(s, 128)],
                                             wv3_sb[:, c, :],
                                             start=(c == 0), stop=(c == 1))

                        vw = smp.tile([128, HEADS + H], F32R, tag="vw")
                        nc.vector.tensor_copy(vw[:, 0:HEADS],
                                              psb[:, 4 * s:4 * s + 4])
                        nc.vector.tensor_tensor(
                            vw[:, HEADS:].rearrange("p (h d) -> p h d", d=D),
                            vep[:].rearrange("p (h d) -> p h d", d=D),
                            psb[:, 4 * s:4 * s + 4][:, :, None].to_broadcast(
                                [128, HEADS, D]),
                            op=ALU.mult)

                        ae = smp.tile([128, 128], F32R, tag="ae")
                        nc.vector.tensor_scalar(
                            ae[:], irow_sb[:],
                            ccols[:, 4 * t + s:4 * t + s + 1], None,
                            ALU.is_equal)

                        sfirst = first and s == 0
                        slast = last and s == SUBT - 1
                        nc.tensor.matmul(acc[:, 0:HEADS + H], ae[:], vw[:],
                                         start=sfirst, stop=slast)

                # ---- group epilogue: normalize, project, store ----
                dene = grpp.tile([128, HEADS], F32)
                nc.vector.tensor_scalar(dene[:], acc[:, 0:HEADS],
                                        1e-30, None, ALU.add)
                rd = grpp.tile([128, HEADS], F32)
                nc.vector.reciprocal(rd[:], dene[:])
                onorm = grpp.tile([128, H], F32)
                for h in range(HEADS):
                    nc.vector.tensor_scalar(onorm[:, ts(h, D)],
                                            acc[:, HEADS + h * D:HEADS + (h + 1) * D],
                                            rd[:, h:h + 1], None, ALU.mult)
                otp = psmisc.tile([128, 2, 128], F32, tag="misc")
                for c in range(2):
                    nc.tensor.transpose(otp[:, c, :], onorm[:, ts(c, 128)],
                                        ident_sb[:])
                otsb = grpp.tile([128, 2, 128], F32R)
                for c in range(2):
                    nc.scalar.activation(otsb[:, c, :], otp[:, c, :],
                                         AF.Identity,
                                         bias=wv3bc_sb[:, c:c + 1], scale=1.0)
                fop = psmisc.tile([128, H], F32, tag="misc")
                for c in range(2):
                    nc.tensor.matmul(fop[:], otsb[:, c, :], wow_sb[:, c, :],
                                     start=(c == 0), stop=(c == 1))
                ofin = grpp.tile([128, H], F32)
                nc.scalar.copy(ofin[:], fop[:])
                nc.sync.dma_start(out_d[g * 128:(g + 1) * 128, :], ofin[:])

    nc.compile()
    return nc


_PROGRAM_CACHE: dict[tuple, "bacc.Bacc"] = {}


def _get_program(T: int, G: int) -> "bacc.Bacc":
    if (T, G) not in _PROGRAM_CACHE:
        _PROGRAM_CACHE[(T, G)] = _build_program(T, G)
    return _PROGRAM_CACHE[(T, G)]


def _greedy_windows(cnt, cap_edges):
    """Split a core's node range into windows of <=128 nodes, <=cap_edges
    edges. Returns list of (node_lo, node_hi) relative to the core."""
    wins = []
    i, N = 0, len(cnt)
    while i < N:
        e = 0
        lo = i
        while i < N and (i - lo) < NODE_BLOCK and e + cnt[i] <= cap_edges:
            e += cnt[i]
            i += 1
        if i == lo:          # single node exceeds cap (cannot happen here)
            i += 1
        wins.append((lo, i))
    return wins


def kernel(h_V, h_E, wv1_w, wv1_b, wv2_w, wv2_b, wv3_w, wv3_b,
           b1_w, b1_b, b2_w, b2_b, b3_w, b3_b, wo_w,
           center_id, batch_id=None, **_unused):
    h_V = np.ascontiguousarray(np.asarray(h_V), dtype=np.float32)
    h_E = np.asarray(h_E)
    center = np.asarray(center_id).astype(np.int64)
    E = center.shape[0]

    # ---- choose tile structure from the data ----
    # per-node edge counts; greedy variable windows (<=128 nodes, <=T*512
    # edges) minimize total tiles G*T vs fixed 128-node groups.
    ncnt_all = np.bincount(center, minlength=NCORES * NODES_PER_CORE)
    best = None
    for T_try in range(4, 13):
        G_try = 0
        wins_try = []
        for k in range(NCORES):
            w = _greedy_windows(
                ncnt_all[k * NODES_PER_CORE:(k + 1) * NODES_PER_CORE],
                T_try * TILE_E)
            wins_try.append(w)
            G_try = max(G_try, len(w))
        key = (G_try * T_try, -T_try)
        if best is None or key < best[0]:
            best = (key, T_try, G_try, wins_try)
    _, T, G, core_wins = best
    EG = T * TILE_E
    EPACK = G * EG
    NPACK = G * NODE_BLOCK

    nc = _get_program(T, G)

    # ---- shared (replicated) weight tensors ----
    b1_w = np.asarray(b1_w, dtype=np.float32)
    shared = {
        "b1we": _round_f32r(b1_w[H:, :]),
        "b1wv": _round_f32r(b1_w[:H, :]),
        "b2w": _round_f32r(np.asarray(b2_w)),
        "b3ws": _round_f32r(np.asarray(b3_w) / np.float32(np.sqrt(D))),
        "wv1": _round_f32r(np.asarray(wv1_w)),
        "wv2": _round_f32r(np.asarray(wv2_w)),
        "wv3": _round_f32r(np.asarray(wv3_w)),
        "wv3bc": np.ascontiguousarray(
            np.asarray(wv3_b, dtype=np.float32).reshape(2, 128).T),
        "wow": _round_f32r(np.asarray(wo_w)),
        "b1b": np.ascontiguousarray(
            np.asarray(b1_b, dtype=np.float32).reshape(2, 128).T),
        "b2b": np.ascontiguousarray(
            np.asarray(b2_b, dtype=np.float32).reshape(2, 128).T),
        "wv1b": np.ascontiguousarray(
            np.asarray(wv1_b, dtype=np.float32).reshape(2, 128).T),
        "wv2b": np.ascontiguousarray(
            np.asarray(wv2_b, dtype=np.float32).reshape(2, 128).T),
        "iota_chan": np.ascontiguousarray(
            np.tile(np.arange(128, dtype=np.float32)[:, None], (1, TILE_E))),
        "iota_row": np.ascontiguousarray(
            np.tile(np.arange(128, dtype=np.float32)[None, :], (128, 1))),
        "ident": np.eye(128, dtype=np.float32),
    }

    hE_r = _round_f32r(np.asarray(h_E, dtype=np.float32))
    hV_r = _round_f32r(h_V)

    ecum = np.concatenate([[0], np.cumsum(ncnt_all)])  # edge offset per node
    in_maps = []
    for k in range(NCORES):
        n0 = k * NODES_PER_CORE
        hvt = np.zeros((H, NPACK), np.float32)
        het = np.zeros((NIN, EPACK), np.float32)
        crel = np.full((EPACK,), -1.0, np.float32)
        for g, (lo, hi) in enumerate(core_wins[k]):
            # h_V.T columns for the window's 128-node span (duplicated
            # across windows; zero-padded past the real node range)
            vhi = min(n0 + lo + NODE_BLOCK, N_NODES)
            if vhi > n0 + lo:
                hvt[:, g * NODE_BLOCK:
                    g * NODE_BLOCK + vhi - n0 - lo] = hV_r[n0 + lo:vhi].T
            e0, e1 = int(ecum[n0 + lo]), int(ecum[n0 + hi])
            cnt = e1 - e0
            if cnt:
                het[:, g * EG:g * EG + cnt] = hE_r[e0:e1].T
                crel[g * EG:g * EG + cnt] = (
                    center[e0:e1] - n0 - lo).astype(np.float32)
        m = dict(shared)
        m["hET"] = het
        m["crel"] = crel
        m["hVT"] = np.ascontiguousarray(hvt)
        in_maps.append(m)

    global _last_run
    _last_run = (in_maps, nc)
    res = bass_utils.run_bass_kernel_spmd(nc, in_maps,
                                          core_ids=list(range(NCORES)))

    full = np.zeros((NCORES * NODES_PER_CORE, H), np.float32)
    for k in range(NCORES):
        n0 = k * NODES_PER_CORE
        o = res.results[k]["out"]
        for g, (lo, hi) in enumerate(core_wins[k]):
            full[n0 + lo:n0 + hi] = o[g * NODE_BLOCK:g * NODE_BLOCK + hi - lo]
    return full[:N_NODES]
